# revision 15
# baseline (speedup 1.0000x reference)
import sys

if "/opt/trn_rl_repo" not in sys.path:
    sys.path.insert(0, "/opt/trn_rl_repo")

import numpy as np

import concourse.bass as bass
import concourse.tile as tile
from concourse import bacc, mybir
from concourse.bass_utils import run_bass_kernel_spmd
from concourse.masks import make_upper_triangular

F32 = mybir.dt.float32
F32R = mybir.dt.float32r
BF16 = mybir.dt.bfloat16
FP16 = mybir.dt.float16

# Problem shape (hardcoded per contract)
B, T, D = 4, 2048, 768
H, HD = 12, 64
N_CORES = 8
HEADS_PER_CORE = 6          # 12 heads / 2 groups
CPC = HEADS_PER_CORE * HD   # 384 qkv columns per core
TC = T // 128               # 16 token tiles of 128
DC = D // 128               # 6 chunks of the model dim
CC = CPC // 128             # 3 chunks of this core's head cols
OC = D // 128               # 6 output-col chunks
WT = 512                    # wide tile / matmul free-dim limit
TW = T // WT                # 4 wide token tiles

# ragged P^T layout: block jc covers i in [jc*128, T); OFF[jc] is its col offset
OFF = [0] * (TC + 1)
for _jc in range(TC):
    OFF[_jc + 1] = OFF[_jc] + (TC - _jc) * 128
PT_COLS = OFF[TC]  # 17408

_CACHE = {}


def _build_nc():
    nc = bacc.Bacc("TRN2", target_bir_lowering=False, debug=False)

    xT = nc.dram_tensor("xT", [D, T], FP16, kind="ExternalInput")
    wqkv = nc.dram_tensor("wqkv", [D, 3 * CPC], FP16, kind="ExternalInput")
    bqk = nc.dram_tensor("bqk", [128, 2 * CC], F32, kind="ExternalInput")
    bv = nc.dram_tensor("bv", [128, CPC], F32, kind="ExternalInput")
    wo = nc.dram_tensor("wo", [CPC, D], FP16, kind="ExternalInput")
    yT = nc.dram_tensor("yT", [D, T], FP16, kind="ExternalOutput")

    with tile.TileContext(nc) as tc:
        with tc.tile_pool(name="persist", bufs=1) as pp:
            qT_sb = pp.tile([128, CC, T], FP16)     # q^T, head cols on partitions
            # k^T stored per head with the partner half zeroed: K=128 matmuls
            # keep the PE activity monitor warm (K=64 runs at half clock)
            kTz_sb = pp.tile([128, HEADS_PER_CORE, T], FP16)
            v_sb = pp.tile([128, TC, HEADS_PER_CORE, HD + 1], FP16)  # v | ones
            attnT_sb = pp.tile([128, CC, T], FP16)  # attention out, [cols, T]
            wo_sb = pp.tile([128, CC, D], FP16)
            bqk_sb = pp.tile([128, 2 * CC], F32)
            bv_sb = pp.tile([128, CPC], F32)
            maskf = pp.tile([128, 128], F32)
            mask01 = pp.tile([128, 128], FP16)
            ones_f = pp.tile([128, HD], F32)
            ones_r = pp.tile([128, HD], F32R)  # lhsT for denominator broadcast

            # partner halves of kTz must be zero (gpsimd; off the DVE/ACT path)
            for h in range(HEADS_PER_CORE):
                if h % 2 == 0:
                    nc.gpsimd.memset(kTz_sb[HD:, h, :], 0.0)
                else:
                    nc.gpsimd.memset(kTz_sb[0:HD, h, :], 0.0)
            nc.gpsimd.memset(v_sb[:, :, :, HD : HD + 1], 1.0)
            nc.gpsimd.memset(ones_f[:], 1.0)
            nc.vector.tensor_copy(ones_r[:], ones_f[:])
            # mask01[j, i] = 1.0 if j <= i else 0.0 (valid causal region, S^T coords)
            make_upper_triangular(nc, maskf, val=1.0, diag=True)
            nc.vector.tensor_copy(mask01[:], maskf[:])

            # ---------------- Phase A: qkv projection ----------------
            with (
                tc.tile_pool(name="loadA", bufs=1) as pA,
                tc.tile_pool(name="psumA", bufs=3, space="PSUM") as psA,
            ):
                xT_sb = pA.tile([128, DC, T], FP16)
                w_sb = pA.tile([128, DC, 3 * CPC], FP16)
                xT_r = xT.ap().rearrange("(o p) t -> p o t", p=128)
                w_r = wqkv.ap().rearrange("(o p) c -> p o c", p=128)

                # startup DMA triggers run ~650ns each; split across SP and
                # ACT queues so the PE can start ~8us in instead of ~18us
                # priming order: v only needs wv (0.6MB) + x, so wv goes
                # per-di first and wq|wk as one bulk transfer; the PE starts
                # on v tiles ~1us after the first chunks land
                nc.sync.dma_start(w_sb[:, 0, 2 * CPC :], w_r[:, 0, 2 * CPC :])
                nc.sync.dma_start(bqk_sb[:], bqk.ap())
                for di in range(1, DC):
                    nc.sync.dma_start(
                        w_sb[:, di, 2 * CPC :], w_r[:, di, 2 * CPC :]
                    )
                nc.sync.dma_start(w_sb[:, :, : 2 * CPC], w_r[:, :, : 2 * CPC])
                x_order = [(0, 0), (0, 1), (0, 2), (0, 3), (0, 4), (0, 5),
                           (1, None), (2, None), (3, None)]
                for tb, di in x_order:
                    if di is not None:
                        nc.scalar.dma_start(
                            xT_sb[:, di, 0:WT], xT_r[:, di, 0:WT]
                        )
                    else:
                        sp = slice(tb * WT, (tb + 1) * WT)
                        nc.scalar.dma_start(xT_sb[:, :, sp], xT_r[:, :, sp])
                nc.sync.dma_start(bv_sb[:], bv.ap())
                nc.sync.dma_start(
                    wo_sb[:], wo.ap().rearrange("(c p) o -> p c o", p=128)
                )

                for tb in range(TW):
                    sp = slice(tb * WT, (tb + 1) * WT)
                    for tj4 in range(WT // 128):  # v token tiles first
                        tj = tb * (WT // 128) + tj4
                        psv = psA.tile([128, CPC], F32, tag="ps_v")
                        for di in range(DC):
                            nc.tensor.matmul(
                                psv[:],
                                xT_sb[:, di, tj * 128 : (tj + 1) * 128],
                                w_sb[:, di, 2 * CPC : 3 * CPC],
                                start=(di == 0),
                                stop=(di == DC - 1),
                            )
                        nc.vector.tensor_add(
                            v_sb[:, tj, :, :HD], psv[:], bv_sb[:]
                        )
                    for hc in range(CC):  # q
                        ps = psA.tile([128, WT], F32, tag="ps_qk")
                        for di in range(DC):
                            nc.tensor.matmul(
                                ps[:],
                                w_sb[:, di, hc * 128 : (hc + 1) * 128],
                                xT_sb[:, di, sp],
                                start=(di == 0),
                                stop=(di == DC - 1),
                            )
                        nc.vector.tensor_scalar_add(
                            qT_sb[:, hc, sp], ps[:], bqk_sb[:, hc : hc + 1]
                        )
                    for hc in range(CC):  # k
                        ps = psA.tile([128, WT], F32, tag="ps_qk")
                        for di in range(DC):
                            nc.tensor.matmul(
                                ps[:],
                                w_sb[:, di, CPC + hc * 128 : CPC + (hc + 1) * 128],
                                xT_sb[:, di, sp],
                                start=(di == 0),
                                stop=(di == DC - 1),
                            )
                        nc.vector.tensor_scalar_add(
                            kTz_sb[0:HD, 2 * hc, sp],
                            ps[0:HD, :],
                            bqk_sb[0:HD, CC + hc : CC + hc + 1],
                        )
                        nc.vector.tensor_scalar_add(
                            kTz_sb[HD:, 2 * hc + 1, sp],
                            ps[HD:, :],
                            bqk_sb[HD:, CC + hc : CC + hc + 1],
                        )

            # ---------- Phase B: causal attention, big-N formulation ----------
            with (
                tc.tile_pool(name="pB", bufs=2) as pB,
                tc.tile_pool(name="outp", bufs=2) as outp,
                tc.tile_pool(name="psumB", bufs=3, space="PSUM") as psB,
            ):
                def score_steps(h, pT):
                    """Generate per-slot closures: S^T matmuls + exp (+ mask)."""
                    hc = h // 2
                    for jc in range(TC):
                        w_cols = (TC - jc) * 128
                        lhsT = kTz_sb[:, h, jc * 128 : (jc + 1) * 128]
                        for s0 in range(0, w_cols, WT):
                            ssz = min(WT, w_cols - s0)

                            def step(jc=jc, s0=s0, ssz=ssz, lhsT=lhsT):
                                st = psB.tile([128, WT], F32, tag="st", bufs=3)
                                nc.tensor.matmul(
                                    st[:, :ssz],
                                    lhsT,
                                    qT_sb[
                                        :,
                                        hc,
                                        jc * 128 + s0 : jc * 128 + s0 + ssz,
                                    ],
                                    start=True,
                                    stop=True,
                                )
                                nc.scalar.activation(
                                    pT[:, OFF[jc] + s0 : OFF[jc] + s0 + ssz],
                                    st[:, :ssz],
                                    mybir.ActivationFunctionType.Exp,
                                )
                                if s0 == 0:
                                    nc.vector.tensor_mul(
                                        pT[:, OFF[jc] : OFF[jc] + 128],
                                        pT[:, OFF[jc] : OFF[jc] + 128],
                                        mask01[:],
                                    )

                            yield step

                def emit_ln(h, q, oTs, lcs):
                    """Early half of the divide: pull the softmax denominator
                    out of PSUM (rounded to f32r for the PE broadcast)."""
                    oT = oTs[q]
                    lc = pB.tile([128, WT], F32R, tag="lc", name=f"lc{h}_{q}")
                    nc.vector.tensor_copy(lc[HD : HD + 1, :], oT[HD : HD + 1, :])
                    lcs[q] = lc

                def emit_div(h, q, oTs, lcs):
                    """Late half: PE broadcasts l to 64 partitions, DVE
                    approx-reciprocals it there and scales the PV output."""
                    hc = h // 2
                    odd = h % 2 == 1
                    i0 = q * WT
                    oT = oTs.pop(q)
                    lc = lcs.pop(q)
                    lP = psB.tile([128, WT], F32, tag="lP", bufs=1)
                    nc.tensor.matmul(
                        lP[0:HD, :],
                        ones_r[HD : HD + 1, :],
                        lc[HD : HD + 1, :],
                        start=True,
                        stop=True,
                    )
                    rcb = pB.tile([128, WT], F32, tag="rcb", name=f"rcb{h}_{q}")
                    nc.vector.reciprocal_approx_fast(rcb[:HD, :], lP[:HD, :])
                    if not odd:
                        nc.vector.tensor_mul(
                            attnT_sb[:HD, hc, i0 : i0 + WT], oT[:HD, :], rcb[:HD, :]
                        )
                    else:
                        tmp = pB.tile([HD, WT], FP16, tag="pvtmp", name=f"pvt{h}_{q}")
                        nc.vector.tensor_mul(tmp[:], oT[:HD, :], rcb[:HD, :])
                        nc.sync.dma_start(attnT_sb[HD:, hc, i0 : i0 + WT], tmp[:])

                def d_steps(tj):
                    """W_o matmuls for one completed 512-wide token block."""
                    ot_box = {}
                    yT_r = yT.ap().rearrange("(o p) t -> p o t", p=128)
                    tsp = slice(tj * WT, (tj + 1) * WT)
                    last_tj = tj == TW - 1
                    for oc in range(OC):

                        def step(oc=oc):
                            if oc == 0:
                                ot_box["t"] = outp.tile(
                                    [128, OC, WT], FP16, tag="ot", name=f"ot{tj}"
                                )
                            ps_wo = psB.tile([128, WT], F32, tag="oT", bufs=3)
                            for dc in range(CC):
                                nc.tensor.matmul(
                                    ps_wo[:],
                                    wo_sb[:, dc, oc * 128 : (oc + 1) * 128],
                                    attnT_sb[:, dc, tsp],
                                    start=(dc == 0),
                                    stop=(dc == CC - 1),
                                )
                            ot = ot_box["t"]
                            nc.vector.tensor_copy(ot[:, oc, :], ps_wo[:])
                            # split the final block's writeback so the tail
                            # only waits on the last two column chunks
                            if last_tj and oc == 3:
                                nc.sync.dma_start(
                                    yT_r[:, 0:4, tsp], ot[:, 0:4, :]
                                )
                            elif oc == OC - 1:
                                if last_tj:
                                    nc.sync.dma_start(
                                        yT_r[:, 4:OC, tsp], ot[:, 4:OC, :]
                                    )
                                else:
                                    nc.sync.dma_start(yT_r[:, :, tsp], ot[:])

                        yield step

                pending = {}  # previous head's final-quarter divide

                def pv_steps(h, pT, d_queue):
                    """PV in transposed form (attnT = (P V)^T / l), as steps.
                    Divides lag one quarter; the final quarter's divide is
                    carried into the NEXT head's stream for slack. The last
                    head enqueues W_o work."""
                    oTs = {}
                    lcs = {}
                    for q in range(TW):
                        i0 = q * WT
                        jhi = min(4 * q + 3, TC - 1)
                        jcs = list(range(jhi + 1))
                        # chunk the accumulation into groups of <=6 matmuls
                        for g0 in range(0, len(jcs), 6):
                            grp = jcs[g0 : g0 + 6]

                            def step(q=q, i0=i0, jhi=jhi, grp=grp, g0=g0):
                                if g0 == 0:
                                    oTs[q] = psB.tile(
                                        [128, WT], F32, tag="oT", bufs=3,
                                        name=f"oT{h}_{q}",
                                    )
                                oT = oTs[q]
                                for jc in grp:
                                    lo = max(jc * 128, i0)
                                    rhs = pT[
                                        :,
                                        OFF[jc] + lo - jc * 128 : OFF[jc]
                                        + i0
                                        + WT
                                        - jc * 128,
                                    ]
                                    nc.tensor.matmul(
                                        oT[: HD + 1, lo - i0 : WT],
                                        v_sb[:, jc, h, :],
                                        rhs,
                                        start=(jc == 0),
                                        stop=(jc == jhi),
                                    )

                            yield step
                            # divide for the previous quarter goes after the
                            # first PV group so the DVE recip chain has slack
                            if g0 == 0:
                                if q == 0 and "d" in pending:
                                    yield pending.pop("d")
                                elif q >= 1:
                                    yield lambda q=q: emit_div(h, q - 1, oTs, lcs)
                        yield lambda q=q: emit_ln(h, q, oTs, lcs)
                        if q >= 1 and d_queue:
                            yield from d_steps(q - 1)
                    if d_queue:
                        yield lambda: emit_div(h, TW - 1, oTs, lcs)
                        yield from d_steps(TW - 1)
                    else:
                        pending["d"] = lambda: emit_div(h, TW - 1, oTs, lcs)

                def interleave(a_steps, b_steps):
                    """Emit steps from both lists, spreading b evenly among a."""
                    a, b = list(a_steps), list(b_steps)
                    if not b:
                        for s in a:
                            s()
                        return
                    ratio = max(1, len(a) // len(b))
                    bi = 0
                    for idx, s in enumerate(a):
                        s()
                        if idx % ratio == ratio - 1 and bi < len(b):
                            b[bi]()
                            bi += 1
                    while bi < len(b):
                        b[bi]()
                        bi += 1

                # odd heads first: their attnT writes go through an SBUF DMA,
                # so the last head (even) feeds W_o without a DMA on the
                # critical path
                order = [1, 3, 5, 0, 2, 4]
                pTs = {}
                h0 = order[0]
                pTs[h0] = pB.tile([128, PT_COLS], FP16, tag="pT", name=f"pT{h0}")
                for s in score_steps(h0, pTs[h0]):
                    s()
                for i in range(1, HEADS_PER_CORE):
                    h, hp = order[i], order[i - 1]
                    pTs[h] = pB.tile([128, PT_COLS], FP16, tag="pT", name=f"pT{h}")
                    interleave(
                        score_steps(h, pTs[h]),
                        pv_steps(hp, pTs[hp], False),
                    )
                    del pTs[hp]
                last = order[-1]
                for s in pv_steps(last, pTs[last], d_queue=True):
                    s()

    nc.compile()
    return nc


def _get_nc():
    if "nc" not in _CACHE:
        _CACHE["nc"] = _build_nc()
    return _CACHE["nc"]


def kernel(x, W_qkv, b_qkv, W_o, b_o, **run_kwargs):
    x = np.asarray(x, dtype=np.float32)
    W_qkv = np.asarray(W_qkv, dtype=np.float32)
    b_qkv = np.asarray(b_qkv, dtype=np.float32)
    W_o = np.asarray(W_o, dtype=np.float32)
    b_o = np.asarray(b_o, dtype=np.float32)

    scale = np.float32(1.0) / np.sqrt(np.float32(HD)).astype(np.float32)

    in_maps = []
    for c in range(N_CORES):
        b = c // 2
        g = c % 2
        cs = g * CPC
        q_sl = slice(cs, cs + CPC)
        k_sl = slice(D + cs, D + cs + CPC)
        v_sl = slice(2 * D + cs, 2 * D + cs + CPC)
        bq = (b_qkv[q_sl] * scale).reshape(CC, 128).T
        bk = b_qkv[k_sl].reshape(CC, 128).T
        in_maps.append(
            {
                "xT": np.ascontiguousarray(x[b].T).astype(np.float16),
                "wqkv": np.concatenate(
                    [W_qkv[:, q_sl] * scale, W_qkv[:, k_sl], W_qkv[:, v_sl]],
                    axis=1,
                ).astype(np.float16),
                "bqk": np.ascontiguousarray(
                    np.concatenate([bq, bk], axis=1).astype(np.float32)
                ),
                "bv": np.ascontiguousarray(
                    np.broadcast_to(b_qkv[v_sl], (128, CPC)).astype(np.float32)
                ),
                "wo": np.ascontiguousarray(W_o[cs : cs + CPC, :]).astype(
                    np.float16
                ),
            }
        )

    nc = _get_nc()
    res = run_bass_kernel_spmd(nc, in_maps, core_ids=list(range(N_CORES)), **run_kwargs)
    _CACHE["last_result"] = res

    out = np.empty((B, T, D), dtype=np.float32)
    for b in range(B):
        acc = res.results[2 * b]["yT"].astype(np.float32) + res.results[
            2 * b + 1
        ]["yT"].astype(np.float32)
        out[b] = acc.T + b_o
    return out
